# revision 32
# baseline (speedup 1.0000x reference)
"""Trainium2 Bass kernel for single-head causal attention.

Problem: B=4, T=4096, C=1024, HD=64 (fp32 inputs).
  q/k/v = x @ W{q,k,v};  scores = q k^T / sqrt(64), causal mask, softmax;
  out = attn @ v.

Sharding (8 cores, SPMD-uniform program):
  core = 2*batch + parity.  The two cores of a batch split the KEY axis into
  interleaved 256-column blocks (even blocks -> parity 0, odd -> parity 1).
  Each core computes, for ALL 4096 queries of its batch, the partial softmax
  numerator (sum_s exp(s_qs) v_s) and denominator (sum_s exp(s_qs)) over its
  own key blocks.  The host sums the two partials and divides.

v4 scheduling model (from v2/v3 trace analysis):
  * The PE queue is strict FIFO; the Tile scheduler's static order ~follows
    emission order.  PE is the most-loaded engine (~48us of matmul), so the
    kernel time ~= PE_busy + PE_stalls.  Two stall sources were fixed:
      - a projection matmul placed before attention work stalls the whole
        queue until its chunk's DMA lands (DMA completion semaphores lag
        ~4-5us behind queue submission; stream runs ~300GB/s from ~8.4us).
        -> projections are emitted in 5 small PIECES per chunk, hand-paced
        so each piece sits in the queue just after the point where its
        input data has landed, and before its consumption deadline.
      - bunched emission put 2-5 whole projection series between a group's
        last scores and the next group's first scores, starving the exp
        stream for ~3.7us at a time.
  * exp tiles are [128,1024] fp32 PSUM pairs (one per key j-block);
    PSUM: scores 2x2 banks + proj 2x[128,512] (2) + out 2x[65,512] (2).
  * Junk warm-up matmuls (6) fill the initial DMA wait and flip the HAM
    clock gate so the first real projections run at 2.4GHz.
  * Diagonal j-block ordered LAST in each group: it is the only consumer
    of chunk g's k/vaug, maximizing projection slack; q(c) gates the next
    group's start so q pieces are emitted before kv pieces.
  * Output evacuation copies run on GpSimd (Pool), chained with the SWDGE
    output DMAs on the same queue, keeping the DVE queue free for the
    projection casts that gate kt/qt readiness.

  Scores are computed transposed (S^T[key, query]) so the PV contraction has
  keys on partitions; softmax max-subtraction is skipped (scores ~ N(0,1),
  exp can't overflow) and the denominator comes from a ones-column appended
  to V (output row 64).  Scores matmuls have K=64 contraction; two key tiles
  are row-packed into the 128x128 PE array and run concurrently.
"""

import os
import sys

import numpy as np

for _p in ("/opt/trn_rl_repo", "/root/.axon_site/_ro/trn_rl_repo"):
    if _p not in sys.path and os.path.isdir(_p):
        sys.path.append(_p)

import ml_dtypes  # noqa: E402

BF16 = ml_dtypes.bfloat16

B, T, C, HD = 4, 4096, 1024, 64
NCORES = 8
NG = 8          # query groups of 512 per batch
GQ = 512        # queries per group
KB = 256        # key block (one pair of 128-key tiles)
NKB = T // KB   # 16 global key blocks, 8 per core
CCH = C // 128  # 8 contraction chunks

_cache = {}


def _build_nc():
    import concourse.bass as bass
    import concourse.mybir as mybir
    import concourse.tile as tile
    from concourse import bacc
    from concourse.bass import ts

    fp32 = mybir.dt.float32
    bf16 = mybir.dt.bfloat16

    nc = bacc.Bacc("TRN2", target_bir_lowering=False, debug=False)

    xT = nc.dram_tensor("xT", [C, T], bf16, kind="ExternalInput")
    wall = nc.dram_tensor("wall", [C, 256], bf16, kind="ExternalInput")  # [Wk|Wv|Wq|Wq]
    maskd = nc.dram_tensor("maskd", [128, 1024], bf16, kind="ExternalInput")
    out_d = nc.dram_tensor("out", [HD + 1, T], fp32, kind="ExternalOutput")

    xT_v = xT[:, :].rearrange("(c p) t -> p c t", p=128)      # [128, 8, T]
    wall_v = wall[:, :].rearrange("(c p) m -> p c m", p=128)  # [128, 8, 256]

    from contextlib import ExitStack

    with tile.TileContext(nc) as tc, ExitStack() as ctx:
        singles = ctx.enter_context(tc.tile_pool(name="singles", bufs=1))
        # ps_s bufs=3: lets the PE run THREE tiles of scores ahead of the exp
        # stream, so a ~0.9us projection piece between tiles no longer stalls
        # ACT (3 x (scores+PV) + piece ~= 3 x 1.16us ACT).  Paid for with
        # single-buffered proj/output accumulators (pieces are spaced >=1
        # tile apart, so the pj braid rarely blocks).
        ps_pj = ctx.enter_context(tc.tile_pool(name="ps_pj", bufs=1, space="PSUM"))
        ps_s = ctx.enter_context(tc.tile_pool(name="ps_s", bufs=3, space="PSUM"))
        ps_o = ctx.enter_context(tc.tile_pool(name="ps_o", bufs=1, space="PSUM"))
        pt_pool = ctx.enter_context(tc.tile_pool(name="pt", bufs=5))
        oe_pool = ctx.enter_context(tc.tile_pool(name="oe", bufs=2))

        # ---- persistent SBUF ----
        xt_sb = singles.tile([128, CCH, T], bf16, tag="xt")           # 64KB/part
        wall_sb = singles.tile([128, CCH, 256], bf16, tag="wall")
        kt_sb = singles.tile([128, T // 2], bf16, tag="kt")           # dup halves
        vt_sb = singles.tile([128, T // 2], bf16, tag="vt")           # rows 64:128
        qt_sb = singles.tile([128, T], bf16, tag="qt")                # dup halves
        vaug_sb = singles.tile([128, T // 2 // 128, HD + 1], bf16, tag="vaug")
        mask_sb = singles.tile([128, 1024], bf16, tag="mask")
        ident_sb = singles.tile([128, 64], bf16, tag="ident")
        junk_sb = singles.tile([128, 640], bf16, tag="junk")

        # ---- input DMAs: [Wk|Wv] half first, then chunk0 halves, so the
        # kv projection can start as early as possible; mask via the idle
        # gpsimd SWDGE queue.  Chunks 0..2 in 512KB halves, 3..7 in 1MB.
        nc.sync.dma_start(out=wall_sb[:, :, 0:128], in_=wall_v[:, :, 0:128])
        nc.sync.dma_start(out=xt_sb[:, :, 0:128], in_=xT_v[:, :, 0:128])
        nc.sync.dma_start(out=xt_sb[:, :, 128:256], in_=xT_v[:, :, 128:256])
        nc.sync.dma_start(out=wall_sb[:, :, 128:256], in_=wall_v[:, :, 128:256])
        nc.sync.dma_start(out=xt_sb[:, :, 256:512], in_=xT_v[:, :, 256:512])
        nc.gpsimd.dma_start(out=mask_sb[:, :], in_=maskd[:, :])
        for hc in range(2, 6):
            nc.sync.dma_start(out=xt_sb[:, :, ts(hc, 256)], in_=xT_v[:, :, ts(hc, 256)])
        for c in range(3, NG):
            nc.sync.dma_start(out=xt_sb[:, :, ts(c, 512)], in_=xT_v[:, :, ts(c, 512)])

        # ---- PE warm-up: 6 junk matmuls fill the ~4us DMA wait and flip
        # the HAM clock gate; they read memset SBUF and rotate through the
        # scores pool (nothing reads them; later scores overwrite).
        nc.vector.memset(junk_sb[:, :], 1.0)
        for w in range(7):
            psj = ps_s.tile([128, 1024], fp32, tag="ss")
            nc.tensor.matmul(
                psj[:, 0:512], lhsT=junk_sb[:, 0:128], rhs=junk_sb[:, 128:640],
                start=True, stop=True,
            )

        # identity (rows 64:128) for PE transpose of V^T tiles
        nc.vector.memset(ident_sb[:, :], 0.0)
        nc.gpsimd.affine_select(
            out=ident_sb[:, :], in_=ident_sb[:, :],
            compare_op=mybir.AluOpType.not_equal, fill=1.0,
            base=-64, pattern=[[-1, 64]], channel_multiplier=1,
        )
        # only the denominator ones-column needs initializing; cols 0:HD are
        # fully written by the V-transpose copies
        nc.vector.memset(vaug_sb[:, :, HD:HD + 1], 1.0)

        def pc_kv(c, split=False):
            # kv projection matmuls + evac casts for the own 256 columns.
            # split=True runs two 128-col accumulation series so the first
            # can start as soon as the first quarter-chunk DMA lands.
            ps = ps_pj.tile([128, 512], fp32, tag="pj", name=f"kv{c}")
            for lo, hi in ([(0, 128), (128, 256)] if split else [(0, 256)]):
                for ch in range(CCH):
                    nc.tensor.matmul(
                        ps[:, lo:hi], lhsT=wall_sb[:, ch, 0:128],
                        rhs=xt_sb[:, ch, 512 * c + lo: 512 * c + hi],
                        start=(ch == 0), stop=(ch == CCH - 1),
                    )
            nc.vector.tensor_copy(out=kt_sb[0:64, ts(c, 256)], in_=ps[0:64, 0:256])
            nc.vector.tensor_copy(out=kt_sb[64:128, ts(c, 256)], in_=ps[0:64, 0:256])
            nc.vector.tensor_copy(out=vt_sb[64:128, ts(c, 256)], in_=ps[64:128, 0:256])

        def pc_tr(c, h):
            # V^T -> V (PE transpose); vaug rows get the ones col from memset
            pst = ps_pj.tile([128, 64], bf16, tag="pj", name=f"tr{c}_{h}")
            nc.tensor.transpose(
                out=pst[:, :],
                in_=vt_sb[64:128, 256 * c + 128 * h: 256 * c + 128 * h + 128],
                identity=ident_sb[64:128, :],
            )
            nc.vector.tensor_copy(out=vaug_sb[:, 2 * c + h, 0:HD], in_=pst[:, :])

        def pc_q(c, half):
            # one 256-query half of the q projection ([Wq|Wq]: dup for free);
            # per-half series keeps DMA gating fine-grained (a 512-col series
            # would stall the PE queue until the whole chunk lands)
            psq = ps_pj.tile([128, 512], fp32, tag="pj", name=f"q{c}_{half}")
            for ch in range(CCH):
                nc.tensor.matmul(
                    psq[:, 256 * half: 256 * half + 256],
                    lhsT=wall_sb[:, ch, 128:256],
                    rhs=xt_sb[:, ch, 512 * c + 256 * half: 512 * c + 256 * half + 256],
                    start=(ch == 0), stop=(ch == CCH - 1),
                )
            nc.vector.tensor_copy(
                out=qt_sb[:, 512 * c + 256 * half: 512 * c + 256 * half + 256],
                in_=psq[:, 256 * half: 256 * half + 256])

        po_t = {}
        pt_of = {}

        def attn_sx(g, j):
            """Scores pair + exp (+ causal mask) for key j-block j of group g."""
            pss = ps_s.tile([128, 1024], fp32, tag="ss")
            for h in range(2):
                nc.tensor.matmul(
                    pss[:, ts(h, 512)],
                    lhsT=kt_sb[64 * h: 64 * h + 64,
                               KB * j + 128 * h: KB * j + 128 * h + 128],
                    rhs=qt_sb[64 * h: 64 * h + 64, ts(g, GQ)],
                    start=True, stop=True,
                )
            pt = pt_pool.tile([128, 1024], bf16, tag="pt")
            nc.scalar.activation(
                out=pt[:, :], in_=pss[:, :],
                func=mybir.ActivationFunctionType.Exp, scale=0.125,
            )
            if j == g:  # diagonal pair: causal mask (parity-specific data)
                nc.vector.tensor_mul(pt[:, :], pt[:, :], mask_sb[:, :])
            pt_of[(g, j)] = pt

        def attn_pv(g, j, last):
            """PV accumulation for (g, j); emitted one tile AFTER its exp so
            a PV waiting on ACT never sits at the PE queue head in front of
            the next tile's (ready) scores matmuls."""
            pt = pt_of.pop((g, j))
            for h in range(2):
                nc.tensor.matmul(
                    po_t[g][:, :],
                    lhsT=vaug_sb[:, 2 * j + h, :],
                    rhs=pt[:, ts(h, 512)],
                    start=(j == 0 and h == 0), stop=(last and h == 1),
                )
            if last:  # group complete: evacuate + stream out
                # PSUM -> SBUF must go through DVE (GpSimd has no PSUM access).
                # Early outputs ride the idle gpsimd SWDGE queue (the sync
                # queue is busy with the input stream until ~38us); the last
                # two use sync HWDGE whose completion receipt is ~1us faster,
                # shortening the end-of-kernel drain.
                oe = oe_pool.tile([HD + 1, 512], fp32, tag="oe")
                nc.vector.tensor_copy(out=oe[:, :], in_=po_t[g][:, :])
                if g == NG - 1:
                    # final output: split across both DMA queues so the two
                    # completion receipts overlap in the end-of-kernel drain
                    nc.sync.dma_start(out=out_d[:, 512 * g: 512 * g + 256],
                                      in_=oe[:, 0:256])
                    nc.gpsimd.dma_start(out=out_d[:, 512 * g + 256: 512 * g + 512],
                                        in_=oe[:, 256:512])
                else:
                    eng = nc.sync if g >= NG - 2 else nc.gpsimd
                    eng.dma_start(out=out_d[:, ts(g, 512)], in_=oe[:, :])

        # ---- emission schedule ----
        # chunk pieces: q half0, q half1, kv, transpose h0, transpose h1.
        # pieces[k] = list of chunk-piece thunks to emit after global attn
        # tile k.  Chunk c (c>=2) data lands at ~13+1.65c us; tile k runs at
        # ~15+1.3k us; chunk c's q must precede group c (tile c(c+1)/2).
        def chunk_pieces(c):
            return [lambda: pc_q(c, 0), lambda: pc_q(c, 1),
                    lambda: pc_kv(c), lambda: pc_tr(c, 0), lambda: pc_tr(c, 1)]

        # One piece per tile: spread so PE never bunches projection work in
        # front of ACT-feeding scores, pushing pieces as LATE as deadlines
        # allow (group c needs q(c) by tile c(c+1)/2, k/vaug only by its
        # diagonal tile c(c+3)/2) -- late groups are ACT-bound with PE slack.
        after_tile = {2: [2, 2, 2, 2, 2], 3: [3, 4, 5, 6, 6],
                      4: [7, 8, 9, 10, 11], 5: [12, 13, 14, 15, 16],
                      6: [17, 18, 19, 20, 21], 7: [22, 23, 24, 25, 26]}
        pieces = {}
        for c, slots in after_tile.items():
            for slot, piece in zip(slots, chunk_pieces(c)):
                pieces.setdefault(slot, []).append(piece)

        # chunk 0: kv first (gates first scores), q after; chunk 1 after
        # group 0's tile (its DMA lands at ~16us; queueing it earlier would
        # stall the PE queue on the DMA semaphore).
        pc_kv(0, split=True)
        pc_q(0, 0)
        pc_q(0, 1)
        pc_tr(0, 0)
        pc_tr(0, 1)

        k = 0            # global attn tile index
        pend = None      # (g, j, last) whose PV is not yet emitted
        for g in range(NG):
            po_t[g] = ps_o.tile([HD + 1, 512], fp32, tag="po", name=f"po{g}")
            # natural order, diagonal last (it is the only consumer of chunk
            # g's k/vaug).  For the FINAL group the diagonal goes second to
            # last so the kernel's serial tail (last exp -> PV -> evac ->
            # DMA) skips the diagonal's mask-multiply.
            order = list(range(g + 1)) if g < NG - 1 else [0, 1, 2, 3, 4, 5, 7, 6]
            for idx, j in enumerate(order):
                attn_sx(g, j)
                if pend is not None:
                    attn_pv(*pend)
                pend = (g, j, idx == g)
                if k == 0:
                    for half in range(2):
                        pc_q(1, half)
                    pc_kv(1)
                    pc_tr(1, 0)
                    pc_tr(1, 1)
                for piece in pieces.get(k, []):
                    piece()
                k += 1
        attn_pv(*pend)

    nc.compile()
    return nc


def _get_nc():
    if "nc" not in _cache:
        _cache["nc"] = _build_nc()
    return _cache["nc"]


def _perm(parity: int) -> np.ndarray:
    # chunk-local order: chunk c = [global block 2c+parity | block 2c+1-parity]
    blocks = np.arange(NKB).reshape(NG, 2)           # [[0,1],[2,3],...]
    if parity == 1:
        blocks = blocks[:, ::-1]
    return (blocks.reshape(-1)[:, None] * KB + np.arange(KB)[None, :]).ravel()


def _mask(parity: int) -> np.ndarray:
    r = np.arange(128)[:, None]
    j = np.arange(KB)[None, :]
    tri0 = (r <= j).astype(np.float32)            # key tile h=0 vs own block
    tri1 = (128 + r <= j).astype(np.float32)      # key tile h=1
    second = np.ones((128, KB), np.float32) if parity == 0 else np.zeros(
        (128, KB), np.float32)
    m = np.concatenate([tri0, second, tri1, second], axis=1)  # [128, 1024]
    return m.astype(BF16)


def _in_maps(x, Wq, Wk, Wv):
    wall = np.concatenate([Wk, Wv, Wq, Wq], axis=1).astype(BF16)
    masks = [_mask(0), _mask(1)]
    perm1 = _perm(1)
    in_maps = []
    for core in range(NCORES):
        b, par = core // 2, core % 2
        xTb = np.ascontiguousarray(x[b].T).astype(BF16)
        xT = xTb if par == 0 else np.ascontiguousarray(xTb[:, perm1])
        in_maps.append({"xT": xT, "wall": wall, "maskd": masks[par]})
    return in_maps


def _combine(outs):
    """outs: 8 arrays [65, T] fp32 -> full [B, T, HD] fp32."""
    full = np.empty((B, T, HD), np.float32)
    for b in range(B):
        oe = outs[2 * b]
        oo = outs[2 * b + 1].reshape(HD + 1, NG, 2, KB)[:, :, ::-1, :].reshape(
            HD + 1, T)
        num = oe[0:HD] + oo[0:HD]
        den = oe[HD] + oo[HD]
        full[b] = (num / den).T
    return full


def run(x, Wq, Wk, Wv, trace=False):
    from concourse.bass_utils import run_bass_kernel_spmd

    nc = _get_nc()
    in_maps = _in_maps(x, Wq, Wk, Wv)
    res = run_bass_kernel_spmd(
        nc, in_maps, core_ids=list(range(NCORES)), trace=trace,
    )
    outs = [r["out"] for r in res.results]
    return _combine(outs), res


def kernel(x, Wq, Wk, Wv, padding_mask=None, **_ignored):
    out, _ = run(np.asarray(x, np.float32), np.asarray(Wq, np.float32),
                 np.asarray(Wk, np.float32), np.asarray(Wv, np.float32))
    return out


# revision 33
# speedup vs baseline: 1.2949x; 1.2949x over previous
"""Trainium2 Bass kernel for single-head causal attention.

Problem: B=4, T=4096, C=1024, HD=64 (fp32 inputs).
  q/k/v = x @ W{q,k,v};  scores = q k^T / sqrt(64), causal mask, softmax;
  out = attn @ v.

Sharding (8 cores, SPMD-uniform program):
  core = 2*batch + parity.  The two cores of a batch split the KEY axis into
  interleaved 256-column blocks (even blocks -> parity 0, odd -> parity 1).
  Each core computes, for ALL 4096 queries of its batch, the partial softmax
  numerator (sum_s exp(s_qs) v_s) and denominator (sum_s exp(s_qs)) over its
  own key blocks.  The host sums the two partials and divides.

v4 scheduling model (from v2/v3 trace analysis):
  * The PE queue is strict FIFO; the Tile scheduler's static order ~follows
    emission order.  PE is the most-loaded engine (~48us of matmul), so the
    kernel time ~= PE_busy + PE_stalls.  Two stall sources were fixed:
      - a projection matmul placed before attention work stalls the whole
        queue until its chunk's DMA lands (DMA completion semaphores lag
        ~4-5us behind queue submission; stream runs ~300GB/s from ~8.4us).
        -> projections are emitted in 5 small PIECES per chunk, hand-paced
        so each piece sits in the queue just after the point where its
        input data has landed, and before its consumption deadline.
      - bunched emission put 2-5 whole projection series between a group's
        last scores and the next group's first scores, starving the exp
        stream for ~3.7us at a time.
  * exp tiles are [128,1024] fp32 PSUM pairs (one per key j-block);
    PSUM: scores 2x2 banks + proj 2x[128,512] (2) + out 2x[65,512] (2).
  * Junk warm-up matmuls (6) fill the initial DMA wait and flip the HAM
    clock gate so the first real projections run at 2.4GHz.
  * Diagonal j-block ordered LAST in each group: it is the only consumer
    of chunk g's k/vaug, maximizing projection slack; q(c) gates the next
    group's start so q pieces are emitted before kv pieces.
  * Output evacuation copies run on GpSimd (Pool), chained with the SWDGE
    output DMAs on the same queue, keeping the DVE queue free for the
    projection casts that gate kt/qt readiness.

  Scores are computed transposed (S^T[key, query]) so the PV contraction has
  keys on partitions; softmax max-subtraction is skipped (scores ~ N(0,1),
  exp can't overflow) and the denominator comes from a ones-column appended
  to V (output row 64).  Scores matmuls have K=64 contraction; two key tiles
  are row-packed into the 128x128 PE array and run concurrently.
"""

import os
import sys

import numpy as np

for _p in ("/opt/trn_rl_repo", "/root/.axon_site/_ro/trn_rl_repo"):
    if _p not in sys.path and os.path.isdir(_p):
        sys.path.append(_p)

import ml_dtypes  # noqa: E402

BF16 = ml_dtypes.bfloat16

B, T, C, HD = 4, 4096, 1024, 64
NCORES = 8
NG = 8          # query groups of 512 per batch
GQ = 512        # queries per group
KB = 256        # key block (one pair of 128-key tiles)
NKB = T // KB   # 16 global key blocks, 8 per core
CCH = C // 128  # 8 contraction chunks

_cache = {}


def _build_nc():
    import concourse.bass as bass
    import concourse.mybir as mybir
    import concourse.tile as tile
    from concourse import bacc
    from concourse.bass import ts

    fp32 = mybir.dt.float32
    bf16 = mybir.dt.bfloat16

    nc = bacc.Bacc("TRN2", target_bir_lowering=False, debug=False)

    xT = nc.dram_tensor("xT", [C, T], bf16, kind="ExternalInput")
    wall = nc.dram_tensor("wall", [C, 256], bf16, kind="ExternalInput")  # [Wk|Wv|Wq|Wq]
    maskd = nc.dram_tensor("maskd", [128, 1024], bf16, kind="ExternalInput")
    out_d = nc.dram_tensor("out", [HD + 1, T], fp32, kind="ExternalOutput")

    xT_v = xT[:, :].rearrange("(c p) t -> p c t", p=128)      # [128, 8, T]
    wall_v = wall[:, :].rearrange("(c p) m -> p c m", p=128)  # [128, 8, 256]

    from contextlib import ExitStack

    with tile.TileContext(nc) as tc, ExitStack() as ctx:
        singles = ctx.enter_context(tc.tile_pool(name="singles", bufs=1))
        # pools: 2/2/2 PSUM buffers (scores 4 banks + proj 2 + out 2 = 8).
        # A ps_s=3 / pj=1 / po=1 variant was tried and REGRESSED ~4us: the
        # single-buffered projection braid (q/kv/pst through one bank) and
        # po-evac serialization cost more than the extra exp lookahead won.
        ps_pj = ctx.enter_context(tc.tile_pool(name="ps_pj", bufs=2, space="PSUM"))
        ps_s = ctx.enter_context(tc.tile_pool(name="ps_s", bufs=2, space="PSUM"))
        ps_o = ctx.enter_context(tc.tile_pool(name="ps_o", bufs=2, space="PSUM"))
        pt_pool = ctx.enter_context(tc.tile_pool(name="pt", bufs=4))
        oe_pool = ctx.enter_context(tc.tile_pool(name="oe", bufs=2))

        # ---- persistent SBUF ----
        xt_sb = singles.tile([128, CCH, T], bf16, tag="xt")           # 64KB/part
        wall_sb = singles.tile([128, CCH, 256], bf16, tag="wall")
        kt_sb = singles.tile([128, T // 2], bf16, tag="kt")           # dup halves
        vt_sb = singles.tile([128, T // 2], bf16, tag="vt")           # rows 64:128
        qt_sb = singles.tile([128, T], bf16, tag="qt")                # dup halves
        vaug_sb = singles.tile([128, T // 2 // 128, HD + 1], bf16, tag="vaug")
        mask_sb = singles.tile([128, 1024], bf16, tag="mask")
        ident_sb = singles.tile([128, 64], bf16, tag="ident")
        junk_sb = singles.tile([128, 640], bf16, tag="junk")

        # ---- input DMAs: [Wk|Wv] half first, then chunk0 halves, so the
        # kv projection can start as early as possible; mask via the idle
        # gpsimd SWDGE queue.  Chunks 0..2 in 512KB halves, 3..7 in 1MB.
        nc.sync.dma_start(out=wall_sb[:, :, 0:128], in_=wall_v[:, :, 0:128])
        nc.sync.dma_start(out=xt_sb[:, :, 0:128], in_=xT_v[:, :, 0:128])
        nc.sync.dma_start(out=xt_sb[:, :, 128:256], in_=xT_v[:, :, 128:256])
        nc.sync.dma_start(out=wall_sb[:, :, 128:256], in_=wall_v[:, :, 128:256])
        nc.sync.dma_start(out=xt_sb[:, :, 256:512], in_=xT_v[:, :, 256:512])
        nc.gpsimd.dma_start(out=mask_sb[:, :], in_=maskd[:, :])
        for hc in range(2, 6):
            nc.sync.dma_start(out=xt_sb[:, :, ts(hc, 256)], in_=xT_v[:, :, ts(hc, 256)])
        for c in range(3, NG):
            nc.sync.dma_start(out=xt_sb[:, :, ts(c, 512)], in_=xT_v[:, :, ts(c, 512)])

        # ---- PE warm-up: 6 junk matmuls fill the ~4us DMA wait and flip
        # the HAM clock gate; they read memset SBUF and rotate through the
        # scores pool (nothing reads them; later scores overwrite).
        nc.vector.memset(junk_sb[:, :], 1.0)
        for w in range(7):
            psj = ps_s.tile([128, 1024], fp32, tag="ss")
            nc.tensor.matmul(
                psj[:, 0:512], lhsT=junk_sb[:, 0:128], rhs=junk_sb[:, 128:640],
                start=True, stop=True,
            )

        # identity (rows 64:128) for PE transpose of V^T tiles
        nc.vector.memset(ident_sb[:, :], 0.0)
        nc.gpsimd.affine_select(
            out=ident_sb[:, :], in_=ident_sb[:, :],
            compare_op=mybir.AluOpType.not_equal, fill=1.0,
            base=-64, pattern=[[-1, 64]], channel_multiplier=1,
        )
        # only the denominator ones-column needs initializing; cols 0:HD are
        # fully written by the V-transpose copies
        nc.vector.memset(vaug_sb[:, :, HD:HD + 1], 1.0)

        def pc_kv(c, split=False):
            # kv projection matmuls + evac casts for the own 256 columns.
            # split=True runs two 128-col accumulation series so the first
            # can start as soon as the first quarter-chunk DMA lands.
            ps = ps_pj.tile([128, 512], fp32, tag="pj", name=f"kv{c}")
            for lo, hi in ([(0, 128), (128, 256)] if split else [(0, 256)]):
                for ch in range(CCH):
                    nc.tensor.matmul(
                        ps[:, lo:hi], lhsT=wall_sb[:, ch, 0:128],
                        rhs=xt_sb[:, ch, 512 * c + lo: 512 * c + hi],
                        start=(ch == 0), stop=(ch == CCH - 1),
                    )
            nc.vector.tensor_copy(out=kt_sb[0:64, ts(c, 256)], in_=ps[0:64, 0:256])
            nc.vector.tensor_copy(out=kt_sb[64:128, ts(c, 256)], in_=ps[0:64, 0:256])
            nc.vector.tensor_copy(out=vt_sb[64:128, ts(c, 256)], in_=ps[64:128, 0:256])

        def pc_tr(c, h):
            # V^T -> V (PE transpose); vaug rows get the ones col from memset
            pst = ps_pj.tile([128, 64], bf16, tag="pj", name=f"tr{c}_{h}")
            nc.tensor.transpose(
                out=pst[:, :],
                in_=vt_sb[64:128, 256 * c + 128 * h: 256 * c + 128 * h + 128],
                identity=ident_sb[64:128, :],
            )
            nc.vector.tensor_copy(out=vaug_sb[:, 2 * c + h, 0:HD], in_=pst[:, :])

        def pc_q(c, half):
            # one 256-query half of the q projection ([Wq|Wq]: dup for free);
            # per-half series keeps DMA gating fine-grained (a 512-col series
            # would stall the PE queue until the whole chunk lands)
            psq = ps_pj.tile([128, 512], fp32, tag="pj", name=f"q{c}_{half}")
            for ch in range(CCH):
                nc.tensor.matmul(
                    psq[:, 256 * half: 256 * half + 256],
                    lhsT=wall_sb[:, ch, 128:256],
                    rhs=xt_sb[:, ch, 512 * c + 256 * half: 512 * c + 256 * half + 256],
                    start=(ch == 0), stop=(ch == CCH - 1),
                )
            nc.vector.tensor_copy(
                out=qt_sb[:, 512 * c + 256 * half: 512 * c + 256 * half + 256],
                in_=psq[:, 256 * half: 256 * half + 256])

        po_t = {}
        pt_of = {}

        def attn_sx(g, j):
            """Scores pair + exp (+ causal mask) for key j-block j of group g."""
            pss = ps_s.tile([128, 1024], fp32, tag="ss")
            for h in range(2):
                nc.tensor.matmul(
                    pss[:, ts(h, 512)],
                    lhsT=kt_sb[64 * h: 64 * h + 64,
                               KB * j + 128 * h: KB * j + 128 * h + 128],
                    rhs=qt_sb[64 * h: 64 * h + 64, ts(g, GQ)],
                    start=True, stop=True,
                )
            pt = pt_pool.tile([128, 1024], bf16, tag="pt")
            nc.scalar.activation(
                out=pt[:, :], in_=pss[:, :],
                func=mybir.ActivationFunctionType.Exp, scale=0.125,
            )
            if j == g:  # diagonal pair: causal mask (parity-specific data)
                nc.vector.tensor_mul(pt[:, :], pt[:, :], mask_sb[:, :])
            pt_of[(g, j)] = pt

        def attn_pv(g, j, last):
            """PV accumulation for (g, j); emitted one tile AFTER its exp so
            a PV waiting on ACT never sits at the PE queue head in front of
            the next tile's (ready) scores matmuls."""
            pt = pt_of.pop((g, j))
            for h in range(2):
                nc.tensor.matmul(
                    po_t[g][:, :],
                    lhsT=vaug_sb[:, 2 * j + h, :],
                    rhs=pt[:, ts(h, 512)],
                    start=(j == 0 and h == 0), stop=(last and h == 1),
                )
            if last:  # group complete: evacuate + stream out
                # PSUM -> SBUF must go through DVE (GpSimd has no PSUM access).
                # Early outputs ride the idle gpsimd SWDGE queue (the sync
                # queue is busy with the input stream until ~38us); the last
                # two use sync HWDGE whose completion receipt is ~1us faster,
                # shortening the end-of-kernel drain.
                oe = oe_pool.tile([HD + 1, 512], fp32, tag="oe")
                nc.vector.tensor_copy(out=oe[:, :], in_=po_t[g][:, :])
                if g == NG - 1:
                    # final output: split across both DMA queues so the two
                    # completion receipts overlap in the end-of-kernel drain
                    nc.sync.dma_start(out=out_d[:, 512 * g: 512 * g + 256],
                                      in_=oe[:, 0:256])
                    nc.gpsimd.dma_start(out=out_d[:, 512 * g + 256: 512 * g + 512],
                                        in_=oe[:, 256:512])
                else:
                    eng = nc.sync if g >= NG - 2 else nc.gpsimd
                    eng.dma_start(out=out_d[:, ts(g, 512)], in_=oe[:, :])

        # ---- emission schedule ----
        # chunk pieces: q half0, q half1, kv, transpose h0, transpose h1.
        # pieces[k] = list of chunk-piece thunks to emit after global attn
        # tile k.  Chunk c (c>=2) data lands at ~13+1.65c us; tile k runs at
        # ~15+1.3k us; chunk c's q must precede group c (tile c(c+1)/2).
        def chunk_pieces(c):
            return [lambda: pc_q(c, 0), lambda: pc_q(c, 1),
                    lambda: pc_kv(c), lambda: pc_tr(c, 0), lambda: pc_tr(c, 1)]

        # One piece per tile: spread so PE never bunches projection work in
        # front of ACT-feeding scores, pushing pieces as LATE as deadlines
        # allow (group c needs q(c) by tile c(c+1)/2, k/vaug only by its
        # diagonal tile c(c+3)/2) -- late groups are ACT-bound with PE slack.
        after_tile = {2: [2, 2, 2, 2, 2], 3: [3, 4, 5, 6, 6],
                      4: [7, 8, 9, 10, 11], 5: [12, 13, 14, 15, 16],
                      6: [17, 18, 19, 20, 21], 7: [22, 23, 24, 25, 26]}
        pieces = {}
        for c, slots in after_tile.items():
            for slot, piece in zip(slots, chunk_pieces(c)):
                pieces.setdefault(slot, []).append(piece)

        # chunk 0: kv first (gates first scores), q after; chunk 1 after
        # group 0's tile (its DMA lands at ~16us; queueing it earlier would
        # stall the PE queue on the DMA semaphore).
        pc_kv(0, split=True)
        pc_q(0, 0)
        pc_q(0, 1)
        pc_tr(0, 0)
        pc_tr(0, 1)

        k = 0            # global attn tile index
        pend = None      # (g, j, last) whose PV is not yet emitted
        for g in range(NG):
            po_t[g] = ps_o.tile([HD + 1, 512], fp32, tag="po", name=f"po{g}")
            # natural order, diagonal last (it is the only consumer of chunk
            # g's k/vaug).  For the FINAL group the diagonal goes second to
            # last so the kernel's serial tail (last exp -> PV -> evac ->
            # DMA) skips the diagonal's mask-multiply.
            order = list(range(g + 1)) if g < NG - 1 else [0, 1, 2, 3, 4, 5, 7, 6]
            for idx, j in enumerate(order):
                attn_sx(g, j)
                if pend is not None:
                    attn_pv(*pend)
                pend = (g, j, idx == g)
                if k == 0:
                    for half in range(2):
                        pc_q(1, half)
                    pc_kv(1)
                    pc_tr(1, 0)
                    pc_tr(1, 1)
                for piece in pieces.get(k, []):
                    piece()
                k += 1
        attn_pv(*pend)

    nc.compile()
    return nc


def _get_nc():
    if "nc" not in _cache:
        _cache["nc"] = _build_nc()
    return _cache["nc"]


def _perm(parity: int) -> np.ndarray:
    # chunk-local order: chunk c = [global block 2c+parity | block 2c+1-parity]
    blocks = np.arange(NKB).reshape(NG, 2)           # [[0,1],[2,3],...]
    if parity == 1:
        blocks = blocks[:, ::-1]
    return (blocks.reshape(-1)[:, None] * KB + np.arange(KB)[None, :]).ravel()


def _mask(parity: int) -> np.ndarray:
    r = np.arange(128)[:, None]
    j = np.arange(KB)[None, :]
    tri0 = (r <= j).astype(np.float32)            # key tile h=0 vs own block
    tri1 = (128 + r <= j).astype(np.float32)      # key tile h=1
    second = np.ones((128, KB), np.float32) if parity == 0 else np.zeros(
        (128, KB), np.float32)
    m = np.concatenate([tri0, second, tri1, second], axis=1)  # [128, 1024]
    return m.astype(BF16)


def _in_maps(x, Wq, Wk, Wv):
    wall = np.concatenate([Wk, Wv, Wq, Wq], axis=1).astype(BF16)
    masks = [_mask(0), _mask(1)]
    perm1 = _perm(1)
    in_maps = []
    for core in range(NCORES):
        b, par = core // 2, core % 2
        xTb = np.ascontiguousarray(x[b].T).astype(BF16)
        xT = xTb if par == 0 else np.ascontiguousarray(xTb[:, perm1])
        in_maps.append({"xT": xT, "wall": wall, "maskd": masks[par]})
    return in_maps


def _combine(outs):
    """outs: 8 arrays [65, T] fp32 -> full [B, T, HD] fp32."""
    full = np.empty((B, T, HD), np.float32)
    for b in range(B):
        oe = outs[2 * b]
        oo = outs[2 * b + 1].reshape(HD + 1, NG, 2, KB)[:, :, ::-1, :].reshape(
            HD + 1, T)
        num = oe[0:HD] + oo[0:HD]
        den = oe[HD] + oo[HD]
        full[b] = (num / den).T
    return full


def run(x, Wq, Wk, Wv, trace=False):
    from concourse.bass_utils import run_bass_kernel_spmd

    nc = _get_nc()
    in_maps = _in_maps(x, Wq, Wk, Wv)
    res = run_bass_kernel_spmd(
        nc, in_maps, core_ids=list(range(NCORES)), trace=trace,
    )
    outs = [r["out"] for r in res.results]
    return _combine(outs), res


def kernel(x, Wq, Wk, Wv, padding_mask=None, **_ignored):
    out, _ = run(np.asarray(x, np.float32), np.asarray(Wq, np.float32),
                 np.asarray(Wk, np.float32), np.asarray(Wv, np.float32))
    return out


# revision 35
# speedup vs baseline: 1.3059x; 1.0085x over previous
"""Trainium2 Bass kernel for single-head causal attention.

Problem: B=4, T=4096, C=1024, HD=64 (fp32 inputs).
  q/k/v = x @ W{q,k,v};  scores = q k^T / sqrt(64), causal mask, softmax;
  out = attn @ v.

Sharding (8 cores, SPMD-uniform program):
  core = 2*batch + parity.  The two cores of a batch split the KEY axis into
  interleaved 256-column blocks (even blocks -> parity 0, odd -> parity 1).
  Each core computes, for ALL 4096 queries of its batch, the partial softmax
  numerator (sum_s exp(s_qs) v_s) and denominator (sum_s exp(s_qs)) over its
  own key blocks.  The host sums the two partials and divides.

v4 scheduling model (from v2/v3 trace analysis):
  * The PE queue is strict FIFO; the Tile scheduler's static order ~follows
    emission order.  PE is the most-loaded engine (~48us of matmul), so the
    kernel time ~= PE_busy + PE_stalls.  Two stall sources were fixed:
      - a projection matmul placed before attention work stalls the whole
        queue until its chunk's DMA lands (DMA completion semaphores lag
        ~4-5us behind queue submission; stream runs ~300GB/s from ~8.4us).
        -> projections are emitted in 5 small PIECES per chunk, hand-paced
        so each piece sits in the queue just after the point where its
        input data has landed, and before its consumption deadline.
      - bunched emission put 2-5 whole projection series between a group's
        last scores and the next group's first scores, starving the exp
        stream for ~3.7us at a time.
  * exp tiles are [128,1024] fp32 PSUM pairs (one per key j-block);
    PSUM: scores 2x2 banks + proj 2x[128,512] (2) + out 2x[65,512] (2).
  * Junk warm-up matmuls (6) fill the initial DMA wait and flip the HAM
    clock gate so the first real projections run at 2.4GHz.
  * Diagonal j-block ordered LAST in each group: it is the only consumer
    of chunk g's k/vaug, maximizing projection slack; q(c) gates the next
    group's start so q pieces are emitted before kv pieces.
  * Output evacuation copies run on GpSimd (Pool), chained with the SWDGE
    output DMAs on the same queue, keeping the DVE queue free for the
    projection casts that gate kt/qt readiness.

  Scores are computed transposed (S^T[key, query]) so the PV contraction has
  keys on partitions; softmax max-subtraction is skipped (scores ~ N(0,1),
  exp can't overflow) and the denominator comes from a ones-column appended
  to V (output row 64).  Scores matmuls have K=64 contraction; two key tiles
  are row-packed into the 128x128 PE array and run concurrently.
"""

import os
import sys

import numpy as np

for _p in ("/opt/trn_rl_repo", "/root/.axon_site/_ro/trn_rl_repo"):
    if _p not in sys.path and os.path.isdir(_p):
        sys.path.append(_p)

import ml_dtypes  # noqa: E402

BF16 = ml_dtypes.bfloat16

B, T, C, HD = 4, 4096, 1024, 64
NCORES = 8
NG = 8          # query groups of 512 per batch
GQ = 512        # queries per group
KB = 256        # key block (one pair of 128-key tiles)
NKB = T // KB   # 16 global key blocks, 8 per core
CCH = C // 128  # 8 contraction chunks

_cache = {}


def _build_nc():
    import concourse.bass as bass
    import concourse.mybir as mybir
    import concourse.tile as tile
    from concourse import bacc
    from concourse.bass import ts

    fp32 = mybir.dt.float32
    bf16 = mybir.dt.bfloat16

    nc = bacc.Bacc("TRN2", target_bir_lowering=False, debug=False)

    xT = nc.dram_tensor("xT", [C, T], bf16, kind="ExternalInput")
    wall = nc.dram_tensor("wall", [C, 256], bf16, kind="ExternalInput")  # [Wk|Wv|Wq|Wq]
    maskd = nc.dram_tensor("maskd", [128, 1024], bf16, kind="ExternalInput")
    out_d = nc.dram_tensor("out", [HD + 1, T], fp32, kind="ExternalOutput")

    xT_v = xT[:, :].rearrange("(c p) t -> p c t", p=128)      # [128, 8, T]
    wall_v = wall[:, :].rearrange("(c p) m -> p c m", p=128)  # [128, 8, 256]

    from contextlib import ExitStack

    with tile.TileContext(nc) as tc, ExitStack() as ctx:
        singles = ctx.enter_context(tc.tile_pool(name="singles", bufs=1))
        ps_pj = ctx.enter_context(tc.tile_pool(name="ps_pj", bufs=2, space="PSUM"))
        ps_s = ctx.enter_context(tc.tile_pool(name="ps_s", bufs=2, space="PSUM"))
        ps_o = ctx.enter_context(tc.tile_pool(name="ps_o", bufs=2, space="PSUM"))
        pt_pool = ctx.enter_context(tc.tile_pool(name="pt", bufs=4))
        oe_pool = ctx.enter_context(tc.tile_pool(name="oe", bufs=2))

        # ---- persistent SBUF ----
        xt_sb = singles.tile([128, CCH, T], bf16, tag="xt")           # 64KB/part
        wall_sb = singles.tile([128, CCH, 256], bf16, tag="wall")
        kt_sb = singles.tile([128, T // 2], bf16, tag="kt")           # dup halves
        vt_sb = singles.tile([128, T // 2], bf16, tag="vt")           # rows 64:128
        qt_sb = singles.tile([128, T], bf16, tag="qt")                # dup halves
        vaug_sb = singles.tile([128, T // 2 // 128, HD + 1], bf16, tag="vaug")
        mask_sb = singles.tile([128, 1024], bf16, tag="mask")
        ident_sb = singles.tile([128, 64], bf16, tag="ident")
        junk_sb = singles.tile([128, 640], bf16, tag="junk")

        # ---- input DMAs: [Wk|Wv] half first, then chunk0 halves, so the
        # kv projection can start as early as possible; mask via the idle
        # gpsimd SWDGE queue.  Chunks 0..2 in 512KB halves, 3..7 in 1MB.
        nc.sync.dma_start(out=wall_sb[:, :, 0:128], in_=wall_v[:, :, 0:128])
        nc.sync.dma_start(out=xt_sb[:, :, 0:128], in_=xT_v[:, :, 0:128])
        nc.sync.dma_start(out=xt_sb[:, :, 128:256], in_=xT_v[:, :, 128:256])
        nc.sync.dma_start(out=wall_sb[:, :, 128:256], in_=wall_v[:, :, 128:256])
        nc.sync.dma_start(out=xt_sb[:, :, 256:512], in_=xT_v[:, :, 256:512])
        nc.gpsimd.dma_start(out=mask_sb[:, :], in_=maskd[:, :])
        for hc in range(2, 6):
            nc.sync.dma_start(out=xt_sb[:, :, ts(hc, 256)], in_=xT_v[:, :, ts(hc, 256)])
        for c in range(3, NG):
            nc.sync.dma_start(out=xt_sb[:, :, ts(c, 512)], in_=xT_v[:, :, ts(c, 512)])

        # ---- PE warm-up: 6 junk matmuls fill the ~4us DMA wait and flip
        # the HAM clock gate; they read memset SBUF and rotate through the
        # scores pool (nothing reads them; later scores overwrite).
        nc.vector.memset(junk_sb[:, :], 1.0)
        for w in range(7):
            psj = ps_s.tile([128, 1024], fp32, tag="ss")
            nc.tensor.matmul(
                psj[:, 0:512], lhsT=junk_sb[:, 0:128], rhs=junk_sb[:, 128:640],
                start=True, stop=True,
            )

        # identity (rows 64:128) for PE transpose of V^T tiles
        nc.vector.memset(ident_sb[:, :], 0.0)
        nc.gpsimd.affine_select(
            out=ident_sb[:, :], in_=ident_sb[:, :],
            compare_op=mybir.AluOpType.not_equal, fill=1.0,
            base=-64, pattern=[[-1, 64]], channel_multiplier=1,
        )
        # only the denominator ones-column needs initializing; cols 0:HD are
        # fully written by the V-transpose copies
        nc.vector.memset(vaug_sb[:, :, HD:HD + 1], 1.0)

        def pc_kv(c, split=False):
            # kv projection matmuls + evac casts for the own 256 columns.
            # split=True runs two 128-col accumulation series so the first
            # can start as soon as the first quarter-chunk DMA lands.
            ps = ps_pj.tile([128, 512], fp32, tag="pj", name=f"kv{c}")
            for lo, hi in ([(0, 128), (128, 256)] if split else [(0, 256)]):
                for ch in range(CCH):
                    nc.tensor.matmul(
                        ps[:, lo:hi], lhsT=wall_sb[:, ch, 0:128],
                        rhs=xt_sb[:, ch, 512 * c + lo: 512 * c + hi],
                        start=(ch == 0), stop=(ch == CCH - 1),
                    )
            nc.vector.tensor_copy(out=kt_sb[0:64, ts(c, 256)], in_=ps[0:64, 0:256])
            nc.vector.tensor_copy(out=kt_sb[64:128, ts(c, 256)], in_=ps[0:64, 0:256])
            nc.vector.tensor_copy(out=vt_sb[64:128, ts(c, 256)], in_=ps[64:128, 0:256])

        def pc_tr(c, h):
            # V^T -> V (PE transpose); vaug rows get the ones col from memset
            pst = ps_pj.tile([128, 64], bf16, tag="pj", name=f"tr{c}_{h}")
            nc.tensor.transpose(
                out=pst[:, :],
                in_=vt_sb[64:128, 256 * c + 128 * h: 256 * c + 128 * h + 128],
                identity=ident_sb[64:128, :],
            )
            nc.vector.tensor_copy(out=vaug_sb[:, 2 * c + h, 0:HD], in_=pst[:, :])

        def pc_q(c, half):
            # one 256-query half of the q projection ([Wq|Wq]: dup for free);
            # per-half series keeps DMA gating fine-grained (a 512-col series
            # would stall the PE queue until the whole chunk lands)
            psq = ps_pj.tile([128, 512], fp32, tag="pj", name=f"q{c}_{half}")
            for ch in range(CCH):
                nc.tensor.matmul(
                    psq[:, 256 * half: 256 * half + 256],
                    lhsT=wall_sb[:, ch, 128:256],
                    rhs=xt_sb[:, ch, 512 * c + 256 * half: 512 * c + 256 * half + 256],
                    start=(ch == 0), stop=(ch == CCH - 1),
                )
            nc.vector.tensor_copy(
                out=qt_sb[:, 512 * c + 256 * half: 512 * c + 256 * half + 256],
                in_=psq[:, 256 * half: 256 * half + 256])

        po_t = {}
        pt_of = {}

        def attn_sx(g, j):
            """Scores pair + exp (+ causal mask) for key j-block j of group g."""
            pss = ps_s.tile([128, 1024], fp32, tag="ss")
            for h in range(2):
                nc.tensor.matmul(
                    pss[:, ts(h, 512)],
                    lhsT=kt_sb[64 * h: 64 * h + 64,
                               KB * j + 128 * h: KB * j + 128 * h + 128],
                    rhs=qt_sb[64 * h: 64 * h + 64, ts(g, GQ)],
                    start=True, stop=True,
                )
            pt = pt_pool.tile([128, 1024], bf16, tag="pt")
            nc.scalar.activation(
                out=pt[:, :], in_=pss[:, :],
                func=mybir.ActivationFunctionType.Exp, scale=0.125,
            )
            if j == g:  # diagonal pair: causal mask (parity-specific data)
                nc.vector.tensor_mul(pt[:, :], pt[:, :], mask_sb[:, :])
            pt_of[(g, j)] = pt

        def attn_pv(g, j):
            """PV accumulation for (g, j); emitted one tile AFTER its exp so
            a PV waiting on ACT never sits at the PE queue head in front of
            the next tile's (ready) scores matmuls."""
            pt = pt_of.pop((g, j))
            for h in range(2):
                nc.tensor.matmul(
                    po_t[g][:, :],
                    lhsT=vaug_sb[:, 2 * j + h, :],
                    rhs=pt[:, ts(h, 512)],
                    start=(j == 0 and h == 0), stop=(j == g and h == 1),
                )
            if j == g:  # group complete: evacuate + stream out
                # PSUM -> SBUF must go through DVE (GpSimd has no PSUM access).
                # Early outputs ride the idle gpsimd SWDGE queue (the sync
                # queue is busy with the input stream until ~38us); the last
                # two use sync HWDGE whose completion receipt is ~1us faster,
                # shortening the end-of-kernel drain.
                oe = oe_pool.tile([HD + 1, 512], fp32, tag="oe")
                nc.vector.tensor_copy(out=oe[:, :], in_=po_t[g][:, :])
                eng = nc.sync if g >= NG - 2 else nc.gpsimd
                eng.dma_start(out=out_d[:, ts(g, 512)], in_=oe[:, :])

        # ---- emission schedule ----
        # chunk pieces: q half0, q half1, kv, transpose h0, transpose h1.
        # pieces[k] = list of chunk-piece thunks to emit after global attn
        # tile k.  Chunk c (c>=2) data lands at ~13+1.65c us; tile k runs at
        # ~15+1.3k us; chunk c's q must precede group c (tile c(c+1)/2).
        def chunk_pieces(c):
            return [lambda: pc_q(c, 0), lambda: pc_q(c, 1),
                    lambda: pc_kv(c), lambda: pc_tr(c, 0), lambda: pc_tr(c, 1)]

        # One piece per tile: spread so PE never bunches projection work in
        # front of ACT-feeding scores, pushing pieces as LATE as deadlines
        # allow (group c needs q(c) by tile c(c+1)/2, k/vaug only by its
        # diagonal tile c(c+3)/2).  The kv/tr pieces of chunks 6-7 sit deep
        # in the ACT-bound final phase (tiles 22-33), where the PE would
        # otherwise idle ~0.3us/tile waiting on the exp stream -- pieces
        # need no scores-PSUM slot, so they run there for free, relieving
        # the mid-kernel tiles whose pieces were stalling ACT.
        after_tile = {2: [2, 2, 2, 2, 2], 3: [3, 4, 5, 6, 6],
                      4: [7, 8, 9, 10, 11], 5: [12, 13, 14, 15, 16],
                      6: [17, 18, 22, 23, 24], 7: [25, 26, 29, 31, 33]}
        pieces = {}
        for c, slots in after_tile.items():
            for slot, piece in zip(slots, chunk_pieces(c)):
                pieces.setdefault(slot, []).append(piece)

        # chunk 0: kv first (gates first scores), q after; chunk 1 after
        # group 0's tile (its DMA lands at ~16us; queueing it earlier would
        # stall the PE queue on the DMA semaphore).
        pc_kv(0, split=True)
        pc_q(0, 0)
        pc_q(0, 1)
        pc_tr(0, 0)
        pc_tr(0, 1)

        k = 0            # global attn tile index
        pend = None      # (g, j) whose PV is not yet emitted
        for g in range(NG):
            po_t[g] = ps_o.tile([HD + 1, 512], fp32, tag="po", name=f"po{g}")
            for j in range(g + 1):  # natural order, diagonal last
                attn_sx(g, j)
                if pend is not None:
                    attn_pv(*pend)
                pend = (g, j)
                if k == 0:
                    for half in range(2):
                        pc_q(1, half)
                    pc_kv(1)
                    pc_tr(1, 0)
                    pc_tr(1, 1)
                for piece in pieces.get(k, []):
                    piece()
                k += 1
        attn_pv(*pend)

    nc.compile()
    return nc


def _get_nc():
    if "nc" not in _cache:
        _cache["nc"] = _build_nc()
    return _cache["nc"]


def _perm(parity: int) -> np.ndarray:
    # chunk-local order: chunk c = [global block 2c+parity | block 2c+1-parity]
    blocks = np.arange(NKB).reshape(NG, 2)           # [[0,1],[2,3],...]
    if parity == 1:
        blocks = blocks[:, ::-1]
    return (blocks.reshape(-1)[:, None] * KB + np.arange(KB)[None, :]).ravel()


def _mask(parity: int) -> np.ndarray:
    r = np.arange(128)[:, None]
    j = np.arange(KB)[None, :]
    tri0 = (r <= j).astype(np.float32)            # key tile h=0 vs own block
    tri1 = (128 + r <= j).astype(np.float32)      # key tile h=1
    second = np.ones((128, KB), np.float32) if parity == 0 else np.zeros(
        (128, KB), np.float32)
    m = np.concatenate([tri0, second, tri1, second], axis=1)  # [128, 1024]
    return m.astype(BF16)


def _in_maps(x, Wq, Wk, Wv):
    wall = np.concatenate([Wk, Wv, Wq, Wq], axis=1).astype(BF16)
    masks = [_mask(0), _mask(1)]
    perm1 = _perm(1)
    in_maps = []
    for core in range(NCORES):
        b, par = core // 2, core % 2
        xTb = np.ascontiguousarray(x[b].T).astype(BF16)
        xT = xTb if par == 0 else np.ascontiguousarray(xTb[:, perm1])
        in_maps.append({"xT": xT, "wall": wall, "maskd": masks[par]})
    return in_maps


def _combine(outs):
    """outs: 8 arrays [65, T] fp32 -> full [B, T, HD] fp32."""
    full = np.empty((B, T, HD), np.float32)
    for b in range(B):
        oe = outs[2 * b]
        oo = outs[2 * b + 1].reshape(HD + 1, NG, 2, KB)[:, :, ::-1, :].reshape(
            HD + 1, T)
        num = oe[0:HD] + oo[0:HD]
        den = oe[HD] + oo[HD]
        full[b] = (num / den).T
    return full


def run(x, Wq, Wk, Wv, trace=False):
    from concourse.bass_utils import run_bass_kernel_spmd

    nc = _get_nc()
    in_maps = _in_maps(x, Wq, Wk, Wv)
    res = run_bass_kernel_spmd(
        nc, in_maps, core_ids=list(range(NCORES)), trace=trace,
    )
    outs = [r["out"] for r in res.results]
    return _combine(outs), res


def kernel(x, Wq, Wk, Wv, padding_mask=None, **_ignored):
    out, _ = run(np.asarray(x, np.float32), np.asarray(Wq, np.float32),
                 np.asarray(Wk, np.float32), np.asarray(Wv, np.float32))
    return out


# revision 36
# speedup vs baseline: 1.3244x; 1.0142x over previous
"""Trainium2 Bass kernel for single-head causal attention.

Problem: B=4, T=4096, C=1024, HD=64 (fp32 inputs).
  q/k/v = x @ W{q,k,v};  scores = q k^T / sqrt(64), causal mask, softmax;
  out = attn @ v.

Sharding (8 cores, SPMD-uniform program):
  core = 2*batch + parity.  The two cores of a batch split the KEY axis into
  interleaved 256-column blocks (even blocks -> parity 0, odd -> parity 1).
  Each core computes, for ALL 4096 queries of its batch, the partial softmax
  numerator (sum_s exp(s_qs) v_s) and denominator (sum_s exp(s_qs)) over its
  own key blocks.  The host sums the two partials and divides.

v4 scheduling model (from v2/v3 trace analysis):
  * The PE queue is strict FIFO; the Tile scheduler's static order ~follows
    emission order.  PE is the most-loaded engine (~48us of matmul), so the
    kernel time ~= PE_busy + PE_stalls.  Two stall sources were fixed:
      - a projection matmul placed before attention work stalls the whole
        queue until its chunk's DMA lands (DMA completion semaphores lag
        ~4-5us behind queue submission; stream runs ~300GB/s from ~8.4us).
        -> projections are emitted in 5 small PIECES per chunk, hand-paced
        so each piece sits in the queue just after the point where its
        input data has landed, and before its consumption deadline.
      - bunched emission put 2-5 whole projection series between a group's
        last scores and the next group's first scores, starving the exp
        stream for ~3.7us at a time.
  * exp tiles are [128,1024] fp32 PSUM pairs (one per key j-block);
    PSUM: scores 2x2 banks + proj 2x[128,512] (2) + out 2x[65,512] (2).
  * Junk warm-up matmuls (6) fill the initial DMA wait and flip the HAM
    clock gate so the first real projections run at 2.4GHz.
  * Diagonal j-block ordered LAST in each group: it is the only consumer
    of chunk g's k/vaug, maximizing projection slack; q(c) gates the next
    group's start so q pieces are emitted before kv pieces.
  * Output evacuation copies run on GpSimd (Pool), chained with the SWDGE
    output DMAs on the same queue, keeping the DVE queue free for the
    projection casts that gate kt/qt readiness.

  Scores are computed transposed (S^T[key, query]) so the PV contraction has
  keys on partitions; softmax max-subtraction is skipped (scores ~ N(0,1),
  exp can't overflow) and the denominator comes from a ones-column appended
  to V (output row 64).  Scores matmuls have K=64 contraction; two key tiles
  are row-packed into the 128x128 PE array and run concurrently.
"""

import os
import sys

import numpy as np

for _p in ("/opt/trn_rl_repo", "/root/.axon_site/_ro/trn_rl_repo"):
    if _p not in sys.path and os.path.isdir(_p):
        sys.path.append(_p)

import ml_dtypes  # noqa: E402

BF16 = ml_dtypes.bfloat16

B, T, C, HD = 4, 4096, 1024, 64
NCORES = 8
NG = 8          # query groups of 512 per batch
GQ = 512        # queries per group
KB = 256        # key block (one pair of 128-key tiles)
NKB = T // KB   # 16 global key blocks, 8 per core
CCH = C // 128  # 8 contraction chunks

_cache = {}


def _build_nc():
    import concourse.bass as bass
    import concourse.mybir as mybir
    import concourse.tile as tile
    from concourse import bacc
    from concourse.bass import ts

    fp32 = mybir.dt.float32
    bf16 = mybir.dt.bfloat16

    nc = bacc.Bacc("TRN2", target_bir_lowering=False, debug=False)

    xT = nc.dram_tensor("xT", [C, T], bf16, kind="ExternalInput")
    wall = nc.dram_tensor("wall", [C, 256], bf16, kind="ExternalInput")  # [Wk|Wv|Wq|Wq]
    maskd = nc.dram_tensor("maskd", [128, 1024], bf16, kind="ExternalInput")
    out_d = nc.dram_tensor("out", [HD + 1, T], fp32, kind="ExternalOutput")

    xT_v = xT[:, :].rearrange("(c p) t -> p c t", p=128)      # [128, 8, T]
    wall_v = wall[:, :].rearrange("(c p) m -> p c m", p=128)  # [128, 8, 256]

    from contextlib import ExitStack

    with tile.TileContext(nc) as tc, ExitStack() as ctx:
        singles = ctx.enter_context(tc.tile_pool(name="singles", bufs=1))
        ps_pj = ctx.enter_context(tc.tile_pool(name="ps_pj", bufs=2, space="PSUM"))
        ps_s = ctx.enter_context(tc.tile_pool(name="ps_s", bufs=2, space="PSUM"))
        ps_o = ctx.enter_context(tc.tile_pool(name="ps_o", bufs=2, space="PSUM"))
        pt_pool = ctx.enter_context(tc.tile_pool(name="pt", bufs=4))
        oe_pool = ctx.enter_context(tc.tile_pool(name="oe", bufs=2))

        # ---- persistent SBUF ----
        xt_sb = singles.tile([128, CCH, T], bf16, tag="xt")           # 64KB/part
        wall_sb = singles.tile([128, CCH, 256], bf16, tag="wall")
        kt_sb = singles.tile([128, T // 2], bf16, tag="kt")           # dup halves
        vt_sb = singles.tile([128, T // 2], bf16, tag="vt")           # rows 64:128
        qt_sb = singles.tile([128, T], bf16, tag="qt")                # dup halves
        vaug_sb = singles.tile([128, T // 2 // 128, HD + 1], bf16, tag="vaug")
        mask_sb = singles.tile([128, 1024], bf16, tag="mask")
        ident_sb = singles.tile([128, 64], bf16, tag="ident")
        junk_sb = singles.tile([128, 640], bf16, tag="junk")

        # ---- input DMAs: [Wk|Wv] half first, then chunk0 halves, so the
        # kv projection can start as early as possible; mask via the idle
        # gpsimd SWDGE queue.  Chunks 0..2 in 512KB halves, 3..7 in 1MB.
        nc.sync.dma_start(out=wall_sb[:, :, 0:128], in_=wall_v[:, :, 0:128])
        nc.sync.dma_start(out=xt_sb[:, :, 0:128], in_=xT_v[:, :, 0:128])
        nc.sync.dma_start(out=xt_sb[:, :, 128:256], in_=xT_v[:, :, 128:256])
        nc.sync.dma_start(out=wall_sb[:, :, 128:256], in_=wall_v[:, :, 128:256])
        nc.sync.dma_start(out=xt_sb[:, :, 256:512], in_=xT_v[:, :, 256:512])
        nc.gpsimd.dma_start(out=mask_sb[:, :], in_=maskd[:, :])
        for hc in range(2, 6):
            nc.sync.dma_start(out=xt_sb[:, :, ts(hc, 256)], in_=xT_v[:, :, ts(hc, 256)])
        for c in range(3, NG):
            nc.sync.dma_start(out=xt_sb[:, :, ts(c, 512)], in_=xT_v[:, :, ts(c, 512)])

        # ---- PE warm-up: 6 junk matmuls fill the ~4us DMA wait and flip
        # the HAM clock gate; they read memset SBUF and rotate through the
        # scores pool (nothing reads them; later scores overwrite).
        nc.vector.memset(junk_sb[:, :], 1.0)
        for w in range(7):
            psj = ps_s.tile([128, 1024], fp32, tag="ss")
            nc.tensor.matmul(
                psj[:, 0:512], lhsT=junk_sb[:, 0:128], rhs=junk_sb[:, 128:640],
                start=True, stop=True,
            )

        # identity (rows 64:128) for PE transpose of V^T tiles
        nc.vector.memset(ident_sb[:, :], 0.0)
        nc.gpsimd.affine_select(
            out=ident_sb[:, :], in_=ident_sb[:, :],
            compare_op=mybir.AluOpType.not_equal, fill=1.0,
            base=-64, pattern=[[-1, 64]], channel_multiplier=1,
        )
        # only the denominator ones-column needs initializing; cols 0:HD are
        # fully written by the V-transpose copies
        nc.vector.memset(vaug_sb[:, :, HD:HD + 1], 1.0)

        def pc_kv(c, split=False):
            # kv projection matmuls + evac casts for the own 256 columns.
            # split=True runs two 128-col accumulation series so the first
            # can start as soon as the first quarter-chunk DMA lands.
            ps = ps_pj.tile([128, 512], fp32, tag="pj", name=f"kv{c}")
            for lo, hi in ([(0, 128), (128, 256)] if split else [(0, 256)]):
                for ch in range(CCH):
                    nc.tensor.matmul(
                        ps[:, lo:hi], lhsT=wall_sb[:, ch, 0:128],
                        rhs=xt_sb[:, ch, 512 * c + lo: 512 * c + hi],
                        start=(ch == 0), stop=(ch == CCH - 1),
                    )
            nc.vector.tensor_copy(out=kt_sb[0:64, ts(c, 256)], in_=ps[0:64, 0:256])
            nc.vector.tensor_copy(out=kt_sb[64:128, ts(c, 256)], in_=ps[0:64, 0:256])
            nc.vector.tensor_copy(out=vt_sb[64:128, ts(c, 256)], in_=ps[64:128, 0:256])

        def pc_tr(c, h):
            # V^T -> V (PE transpose); vaug rows get the ones col from memset
            pst = ps_pj.tile([128, 64], bf16, tag="pj", name=f"tr{c}_{h}")
            nc.tensor.transpose(
                out=pst[:, :],
                in_=vt_sb[64:128, 256 * c + 128 * h: 256 * c + 128 * h + 128],
                identity=ident_sb[64:128, :],
            )
            nc.vector.tensor_copy(out=vaug_sb[:, 2 * c + h, 0:HD], in_=pst[:, :])

        def pc_q(c, half):
            # one 256-query half of the q projection ([Wq|Wq]: dup for free);
            # per-half series keeps DMA gating fine-grained (a 512-col series
            # would stall the PE queue until the whole chunk lands)
            psq = ps_pj.tile([128, 512], fp32, tag="pj", name=f"q{c}_{half}")
            for ch in range(CCH):
                nc.tensor.matmul(
                    psq[:, 256 * half: 256 * half + 256],
                    lhsT=wall_sb[:, ch, 128:256],
                    rhs=xt_sb[:, ch, 512 * c + 256 * half: 512 * c + 256 * half + 256],
                    start=(ch == 0), stop=(ch == CCH - 1),
                )
            nc.vector.tensor_copy(
                out=qt_sb[:, 512 * c + 256 * half: 512 * c + 256 * half + 256],
                in_=psq[:, 256 * half: 256 * half + 256])

        po_t = {}
        pt_of = {}

        def attn_sx(g, j):
            """Scores pair + exp (+ causal mask) for key j-block j of group g."""
            pss = ps_s.tile([128, 1024], fp32, tag="ss")
            for h in range(2):
                nc.tensor.matmul(
                    pss[:, ts(h, 512)],
                    lhsT=kt_sb[64 * h: 64 * h + 64,
                               KB * j + 128 * h: KB * j + 128 * h + 128],
                    rhs=qt_sb[64 * h: 64 * h + 64, ts(g, GQ)],
                    start=True, stop=True,
                )
            pt = pt_pool.tile([128, 1024], bf16, tag="pt")
            nc.scalar.activation(
                out=pt[:, :], in_=pss[:, :],
                func=mybir.ActivationFunctionType.Exp, scale=0.125,
            )
            if j == g:  # diagonal pair: causal mask (parity-specific data)
                nc.vector.tensor_mul(pt[:, :], pt[:, :], mask_sb[:, :])
            pt_of[(g, j)] = pt

        def attn_pv(g, j):
            """PV accumulation for (g, j); emitted one tile AFTER its exp so
            a PV waiting on ACT never sits at the PE queue head in front of
            the next tile's (ready) scores matmuls."""
            pt = pt_of.pop((g, j))
            for h in range(2):
                nc.tensor.matmul(
                    po_t[g][:, :],
                    lhsT=vaug_sb[:, 2 * j + h, :],
                    rhs=pt[:, ts(h, 512)],
                    start=(j == 0 and h == 0), stop=(j == g and h == 1),
                )
            if j == g:  # group complete: evacuate + stream out
                # PSUM -> SBUF must go through DVE (GpSimd has no PSUM access).
                # Early outputs ride the idle gpsimd SWDGE queue (the sync
                # queue is busy with the input stream until ~38us); the last
                # two use sync HWDGE whose completion receipt is ~1us faster,
                # shortening the end-of-kernel drain.
                oe = oe_pool.tile([HD + 1, 512], fp32, tag="oe")
                nc.vector.tensor_copy(out=oe[:, :], in_=po_t[g][:, :])
                eng = nc.sync if g >= NG - 2 else nc.gpsimd
                eng.dma_start(out=out_d[:, ts(g, 512)], in_=oe[:, :])

        # ---- emission schedule ----
        # chunk pieces: q half0, q half1, kv, transpose h0, transpose h1.
        # pieces[k] = list of chunk-piece thunks to emit after global attn
        # tile k.  Chunk c (c>=2) data lands at ~13+1.65c us; tile k runs at
        # ~15+1.3k us; chunk c's q must precede group c (tile c(c+1)/2).
        def chunk_pieces(c):
            return [lambda: pc_q(c, 0), lambda: pc_q(c, 1),
                    lambda: pc_kv(c), lambda: pc_tr(c, 0), lambda: pc_tr(c, 1)]

        # One piece per tile: spread so PE never bunches projection work in
        # front of ACT-feeding scores, pushing pieces as LATE as deadlines
        # allow (group c needs q(c) by tile c(c+1)/2, k/vaug only by its
        # diagonal tile c(c+3)/2).  The kv/tr pieces of chunks 6-7 sit deep
        # in the ACT-bound final phase (tiles 22-33), where the PE would
        # otherwise idle ~0.3us/tile waiting on the exp stream -- pieces
        # need no scores-PSUM slot, so they run there for free, relieving
        # the mid-kernel tiles whose pieces were stalling ACT.
        # chunk 2's kv/tr pieces go AFTER tile 3: only q(2) gates g2's first
        # scores (tile 3), and the FIFO queue otherwise makes them delay it
        # (kt(2) deadline is tile 5, vaug(2) tile 6 -- still met from 3/4).
        after_tile = {2: [2, 2, 3, 3, 4], 3: [3, 4, 5, 6, 6],
                      4: [7, 8, 9, 10, 11], 5: [12, 13, 14, 15, 16],
                      6: [17, 18, 22, 23, 24], 7: [25, 26, 29, 31, 33]}
        pieces = {}
        for c, slots in after_tile.items():
            for slot, piece in zip(slots, chunk_pieces(c)):
                pieces.setdefault(slot, []).append(piece)

        # chunk 0: kv first (gates first scores), q after; chunk 1 after
        # group 0's tile (its DMA lands at ~16us; queueing it earlier would
        # stall the PE queue on the DMA semaphore).
        pc_kv(0, split=True)
        pc_q(0, 0)
        pc_q(0, 1)
        pc_tr(0, 0)
        pc_tr(0, 1)

        k = 0            # global attn tile index
        pend = None      # (g, j) whose PV is not yet emitted
        for g in range(NG):
            po_t[g] = ps_o.tile([HD + 1, 512], fp32, tag="po", name=f"po{g}")
            for j in range(g + 1):  # natural order, diagonal last
                attn_sx(g, j)
                if pend is not None:
                    attn_pv(*pend)
                pend = (g, j)
                if k == 0:
                    for half in range(2):
                        pc_q(1, half)
                    pc_kv(1)
                    pc_tr(1, 0)
                    pc_tr(1, 1)
                for piece in pieces.get(k, []):
                    piece()
                k += 1
        attn_pv(*pend)

    nc.compile()
    return nc


def _get_nc():
    if "nc" not in _cache:
        _cache["nc"] = _build_nc()
    return _cache["nc"]


def _perm(parity: int) -> np.ndarray:
    # chunk-local order: chunk c = [global block 2c+parity | block 2c+1-parity]
    blocks = np.arange(NKB).reshape(NG, 2)           # [[0,1],[2,3],...]
    if parity == 1:
        blocks = blocks[:, ::-1]
    return (blocks.reshape(-1)[:, None] * KB + np.arange(KB)[None, :]).ravel()


def _mask(parity: int) -> np.ndarray:
    r = np.arange(128)[:, None]
    j = np.arange(KB)[None, :]
    tri0 = (r <= j).astype(np.float32)            # key tile h=0 vs own block
    tri1 = (128 + r <= j).astype(np.float32)      # key tile h=1
    second = np.ones((128, KB), np.float32) if parity == 0 else np.zeros(
        (128, KB), np.float32)
    m = np.concatenate([tri0, second, tri1, second], axis=1)  # [128, 1024]
    return m.astype(BF16)


def _in_maps(x, Wq, Wk, Wv):
    wall = np.concatenate([Wk, Wv, Wq, Wq], axis=1).astype(BF16)
    masks = [_mask(0), _mask(1)]
    perm1 = _perm(1)
    in_maps = []
    for core in range(NCORES):
        b, par = core // 2, core % 2
        xTb = np.ascontiguousarray(x[b].T).astype(BF16)
        xT = xTb if par == 0 else np.ascontiguousarray(xTb[:, perm1])
        in_maps.append({"xT": xT, "wall": wall, "maskd": masks[par]})
    return in_maps


def _combine(outs):
    """outs: 8 arrays [65, T] fp32 -> full [B, T, HD] fp32."""
    full = np.empty((B, T, HD), np.float32)
    for b in range(B):
        oe = outs[2 * b]
        oo = outs[2 * b + 1].reshape(HD + 1, NG, 2, KB)[:, :, ::-1, :].reshape(
            HD + 1, T)
        num = oe[0:HD] + oo[0:HD]
        den = oe[HD] + oo[HD]
        full[b] = (num / den).T
    return full


def run(x, Wq, Wk, Wv, trace=False):
    from concourse.bass_utils import run_bass_kernel_spmd

    nc = _get_nc()
    in_maps = _in_maps(x, Wq, Wk, Wv)
    res = run_bass_kernel_spmd(
        nc, in_maps, core_ids=list(range(NCORES)), trace=trace,
    )
    outs = [r["out"] for r in res.results]
    return _combine(outs), res


def kernel(x, Wq, Wk, Wv, padding_mask=None, **_ignored):
    out, _ = run(np.asarray(x, np.float32), np.asarray(Wq, np.float32),
                 np.asarray(Wk, np.float32), np.asarray(Wv, np.float32))
    return out


# revision 37
# speedup vs baseline: 1.3317x; 1.0055x over previous
"""Trainium2 Bass kernel for single-head causal attention.

Problem: B=4, T=4096, C=1024, HD=64 (fp32 inputs).
  q/k/v = x @ W{q,k,v};  scores = q k^T / sqrt(64), causal mask, softmax;
  out = attn @ v.

Sharding (8 cores, SPMD-uniform program):
  core = 2*batch + parity.  The two cores of a batch split the KEY axis into
  interleaved 256-column blocks (even blocks -> parity 0, odd -> parity 1).
  Each core computes, for ALL 4096 queries of its batch, the partial softmax
  numerator (sum_s exp(s_qs) v_s) and denominator (sum_s exp(s_qs)) over its
  own key blocks.  The host sums the two partials and divides.

v4 scheduling model (from v2/v3 trace analysis):
  * The PE queue is strict FIFO; the Tile scheduler's static order ~follows
    emission order.  PE is the most-loaded engine (~48us of matmul), so the
    kernel time ~= PE_busy + PE_stalls.  Two stall sources were fixed:
      - a projection matmul placed before attention work stalls the whole
        queue until its chunk's DMA lands (DMA completion semaphores lag
        ~4-5us behind queue submission; stream runs ~300GB/s from ~8.4us).
        -> projections are emitted in 5 small PIECES per chunk, hand-paced
        so each piece sits in the queue just after the point where its
        input data has landed, and before its consumption deadline.
      - bunched emission put 2-5 whole projection series between a group's
        last scores and the next group's first scores, starving the exp
        stream for ~3.7us at a time.
  * exp tiles are [128,1024] fp32 PSUM pairs (one per key j-block);
    PSUM: scores 2x2 banks + proj 2x[128,512] (2) + out 2x[65,512] (2).
  * Junk warm-up matmuls (6) fill the initial DMA wait and flip the HAM
    clock gate so the first real projections run at 2.4GHz.
  * Diagonal j-block ordered LAST in each group: it is the only consumer
    of chunk g's k/vaug, maximizing projection slack; q(c) gates the next
    group's start so q pieces are emitted before kv pieces.
  * Output evacuation copies run on GpSimd (Pool), chained with the SWDGE
    output DMAs on the same queue, keeping the DVE queue free for the
    projection casts that gate kt/qt readiness.

  Scores are computed transposed (S^T[key, query]) so the PV contraction has
  keys on partitions; softmax max-subtraction is skipped (scores ~ N(0,1),
  exp can't overflow) and the denominator comes from a ones-column appended
  to V (output row 64).  Scores matmuls have K=64 contraction; two key tiles
  are row-packed into the 128x128 PE array and run concurrently.
"""

import os
import sys

import numpy as np

for _p in ("/opt/trn_rl_repo", "/root/.axon_site/_ro/trn_rl_repo"):
    if _p not in sys.path and os.path.isdir(_p):
        sys.path.append(_p)

import ml_dtypes  # noqa: E402

BF16 = ml_dtypes.bfloat16

B, T, C, HD = 4, 4096, 1024, 64
NCORES = 8
NG = 8          # query groups of 512 per batch
GQ = 512        # queries per group
KB = 256        # key block (one pair of 128-key tiles)
NKB = T // KB   # 16 global key blocks, 8 per core
CCH = C // 128  # 8 contraction chunks

_cache = {}


def _build_nc():
    import concourse.bass as bass
    import concourse.mybir as mybir
    import concourse.tile as tile
    from concourse import bacc
    from concourse.bass import ts

    fp32 = mybir.dt.float32
    bf16 = mybir.dt.bfloat16

    nc = bacc.Bacc("TRN2", target_bir_lowering=False, debug=False)

    xT = nc.dram_tensor("xT", [C, T], bf16, kind="ExternalInput")
    wall = nc.dram_tensor("wall", [C, 256], bf16, kind="ExternalInput")  # [Wk|Wv|Wq|Wq]
    maskd = nc.dram_tensor("maskd", [128, 1024], bf16, kind="ExternalInput")
    out_d = nc.dram_tensor("out", [HD + 1, T], fp32, kind="ExternalOutput")

    xT_v = xT[:, :].rearrange("(c p) t -> p c t", p=128)      # [128, 8, T]
    wall_v = wall[:, :].rearrange("(c p) m -> p c m", p=128)  # [128, 8, 256]

    from contextlib import ExitStack

    with tile.TileContext(nc) as tc, ExitStack() as ctx:
        singles = ctx.enter_context(tc.tile_pool(name="singles", bufs=1))
        ps_pj = ctx.enter_context(tc.tile_pool(name="ps_pj", bufs=2, space="PSUM"))
        ps_s = ctx.enter_context(tc.tile_pool(name="ps_s", bufs=2, space="PSUM"))
        ps_o = ctx.enter_context(tc.tile_pool(name="ps_o", bufs=2, space="PSUM"))
        pt_pool = ctx.enter_context(tc.tile_pool(name="pt", bufs=4))
        oe_pool = ctx.enter_context(tc.tile_pool(name="oe", bufs=2))

        # ---- persistent SBUF ----
        xt_sb = singles.tile([128, CCH, T], bf16, tag="xt")           # 64KB/part
        wall_sb = singles.tile([128, CCH, 256], bf16, tag="wall")
        kt_sb = singles.tile([128, T // 2], bf16, tag="kt")           # dup halves
        vt_sb = singles.tile([128, T // 2], bf16, tag="vt")           # rows 64:128
        qt_sb = singles.tile([128, T], bf16, tag="qt")                # dup halves
        vaug_sb = singles.tile([128, T // 2 // 128, HD + 1], bf16, tag="vaug")
        mask_sb = singles.tile([128, 1024], bf16, tag="mask")
        ident_sb = singles.tile([128, 64], bf16, tag="ident")
        junk_sb = singles.tile([128, 640], bf16, tag="junk")

        # ---- input DMAs: [Wk|Wv] half first, then chunk0 halves, so the
        # kv projection can start as early as possible; mask via the idle
        # gpsimd SWDGE queue.  Chunks 0..2 in 512KB halves, 3..7 in 1MB.
        nc.sync.dma_start(out=wall_sb[:, :, 0:128], in_=wall_v[:, :, 0:128])
        nc.sync.dma_start(out=xt_sb[:, :, 0:128], in_=xT_v[:, :, 0:128])
        nc.sync.dma_start(out=xt_sb[:, :, 128:256], in_=xT_v[:, :, 128:256])
        nc.sync.dma_start(out=wall_sb[:, :, 128:256], in_=wall_v[:, :, 128:256])
        nc.sync.dma_start(out=xt_sb[:, :, 256:512], in_=xT_v[:, :, 256:512])
        nc.gpsimd.dma_start(out=mask_sb[:, :], in_=maskd[:, :])
        for hc in range(2, 6):
            nc.sync.dma_start(out=xt_sb[:, :, ts(hc, 256)], in_=xT_v[:, :, ts(hc, 256)])
        for c in range(3, NG):
            nc.sync.dma_start(out=xt_sb[:, :, ts(c, 512)], in_=xT_v[:, :, ts(c, 512)])

        # ---- PE warm-up: 6 junk matmuls fill the ~4us DMA wait and flip
        # the HAM clock gate; they read memset SBUF and rotate through the
        # scores pool (nothing reads them; later scores overwrite).
        nc.vector.memset(junk_sb[:, :], 1.0)
        for w in range(7):
            psj = ps_s.tile([128, 1024], fp32, tag="ss")
            nc.tensor.matmul(
                psj[:, 0:512], lhsT=junk_sb[:, 0:128], rhs=junk_sb[:, 128:640],
                start=True, stop=True,
            )

        # identity (rows 64:128) for PE transpose of V^T tiles
        nc.vector.memset(ident_sb[:, :], 0.0)
        nc.gpsimd.affine_select(
            out=ident_sb[:, :], in_=ident_sb[:, :],
            compare_op=mybir.AluOpType.not_equal, fill=1.0,
            base=-64, pattern=[[-1, 64]], channel_multiplier=1,
        )
        # only the denominator ones-column needs initializing; cols 0:HD are
        # fully written by the V-transpose copies
        nc.vector.memset(vaug_sb[:, :, HD:HD + 1], 1.0)

        def pc_kv(c, split=False):
            # kv projection matmuls + evac casts for the own 256 columns.
            # split=True runs two 128-col accumulation series so the first
            # can start as soon as the first quarter-chunk DMA lands.
            ps = ps_pj.tile([128, 512], fp32, tag="pj", name=f"kv{c}")
            for lo, hi in ([(0, 128), (128, 256)] if split else [(0, 256)]):
                for ch in range(CCH):
                    nc.tensor.matmul(
                        ps[:, lo:hi], lhsT=wall_sb[:, ch, 0:128],
                        rhs=xt_sb[:, ch, 512 * c + lo: 512 * c + hi],
                        start=(ch == 0), stop=(ch == CCH - 1),
                    )
            nc.vector.tensor_copy(out=kt_sb[0:64, ts(c, 256)], in_=ps[0:64, 0:256])
            nc.vector.tensor_copy(out=kt_sb[64:128, ts(c, 256)], in_=ps[0:64, 0:256])
            nc.vector.tensor_copy(out=vt_sb[64:128, ts(c, 256)], in_=ps[64:128, 0:256])

        def pc_tr(c, h):
            # V^T -> V (PE transpose); vaug rows get the ones col from memset
            pst = ps_pj.tile([128, 64], bf16, tag="pj", name=f"tr{c}_{h}")
            nc.tensor.transpose(
                out=pst[:, :],
                in_=vt_sb[64:128, 256 * c + 128 * h: 256 * c + 128 * h + 128],
                identity=ident_sb[64:128, :],
            )
            nc.vector.tensor_copy(out=vaug_sb[:, 2 * c + h, 0:HD], in_=pst[:, :])

        def pc_q(c, half):
            # one 256-query half of the q projection ([Wq|Wq]: dup for free);
            # per-half series keeps DMA gating fine-grained (a 512-col series
            # would stall the PE queue until the whole chunk lands)
            psq = ps_pj.tile([128, 512], fp32, tag="pj", name=f"q{c}_{half}")
            for ch in range(CCH):
                nc.tensor.matmul(
                    psq[:, 256 * half: 256 * half + 256],
                    lhsT=wall_sb[:, ch, 128:256],
                    rhs=xt_sb[:, ch, 512 * c + 256 * half: 512 * c + 256 * half + 256],
                    start=(ch == 0), stop=(ch == CCH - 1),
                )
            nc.vector.tensor_copy(
                out=qt_sb[:, 512 * c + 256 * half: 512 * c + 256 * half + 256],
                in_=psq[:, 256 * half: 256 * half + 256])

        po_t = {}
        pt_of = {}

        def attn_sx(g, j):
            """Scores pair + exp (+ causal mask) for key j-block j of group g."""
            pss = ps_s.tile([128, 1024], fp32, tag="ss")
            for h in range(2):
                nc.tensor.matmul(
                    pss[:, ts(h, 512)],
                    lhsT=kt_sb[64 * h: 64 * h + 64,
                               KB * j + 128 * h: KB * j + 128 * h + 128],
                    rhs=qt_sb[64 * h: 64 * h + 64, ts(g, GQ)],
                    start=True, stop=True,
                )
            pt = pt_pool.tile([128, 1024], bf16, tag="pt")
            nc.scalar.activation(
                out=pt[:, :], in_=pss[:, :],
                func=mybir.ActivationFunctionType.Exp, scale=0.125,
            )
            if j == g:  # diagonal pair: causal mask (parity-specific data)
                nc.vector.tensor_mul(pt[:, :], pt[:, :], mask_sb[:, :])
            pt_of[(g, j)] = pt

        def attn_pv(g, j):
            """PV accumulation for (g, j); emitted one tile AFTER its exp so
            a PV waiting on ACT never sits at the PE queue head in front of
            the next tile's (ready) scores matmuls."""
            pt = pt_of.pop((g, j))
            for h in range(2):
                nc.tensor.matmul(
                    po_t[g][:, :],
                    lhsT=vaug_sb[:, 2 * j + h, :],
                    rhs=pt[:, ts(h, 512)],
                    start=(j == 0 and h == 0), stop=(j == g and h == 1),
                )
            if j == g:  # group complete: evacuate + stream out
                # PSUM -> SBUF must go through DVE (GpSimd has no PSUM access).
                # Early outputs ride the idle gpsimd SWDGE queue (the sync
                # queue is busy with the input stream until ~38us); the last
                # two use sync HWDGE whose completion receipt is ~1us faster,
                # shortening the end-of-kernel drain.
                oe = oe_pool.tile([HD + 1, 512], fp32, tag="oe")
                nc.vector.tensor_copy(out=oe[:, :], in_=po_t[g][:, :])
                eng = nc.sync if g >= NG - 2 else nc.gpsimd
                eng.dma_start(out=out_d[:, ts(g, 512)], in_=oe[:, :])

        # ---- emission schedule ----
        # chunk pieces: q half0, q half1, kv, transpose h0, transpose h1.
        # pieces[k] = list of chunk-piece thunks to emit after global attn
        # tile k.  Chunk c (c>=2) data lands at ~13+1.65c us; tile k runs at
        # ~15+1.3k us; chunk c's q must precede group c (tile c(c+1)/2).
        def chunk_pieces(c):
            return [lambda: pc_q(c, 0), lambda: pc_q(c, 1),
                    lambda: pc_kv(c), lambda: pc_tr(c, 0), lambda: pc_tr(c, 1)]

        # One piece per tile: spread so PE never bunches projection work in
        # front of ACT-feeding scores, pushing pieces as LATE as deadlines
        # allow (group c needs q(c) by tile c(c+1)/2, k/vaug only by its
        # diagonal tile c(c+3)/2).  The kv/tr pieces of chunks 6-7 sit deep
        # in the ACT-bound final phase (tiles 22-33), where the PE would
        # otherwise idle ~0.3us/tile waiting on the exp stream -- pieces
        # need no scores-PSUM slot, so they run there for free, relieving
        # the mid-kernel tiles whose pieces were stalling ACT.
        # Each chunk's kv/tr pieces go AFTER its group-start tile: only q(c)
        # gates group c's first scores (tile c(c+1)/2), and the FIFO queue
        # otherwise makes kv/tr delay it; kt(c) is only needed at the
        # group's diagonal (tile c(c+3)/2) and vaug(c) one tile later, so
        # the slid placements all keep >=2-tile margins.
        after_tile = {2: [2, 2, 3, 3, 4], 3: [3, 4, 6, 6, 7],
                      4: [7, 8, 10, 11, 12], 5: [12, 13, 15, 16, 17],
                      6: [18, 19, 22, 23, 24], 7: [25, 26, 29, 31, 33]}
        pieces = {}
        for c, slots in after_tile.items():
            for slot, piece in zip(slots, chunk_pieces(c)):
                pieces.setdefault(slot, []).append(piece)

        # chunk 0: kv first (gates first scores), q after; chunk 1 after
        # group 0's tile (its DMA lands at ~16us; queueing it earlier would
        # stall the PE queue on the DMA semaphore).
        pc_kv(0, split=True)
        pc_q(0, 0)
        pc_q(0, 1)
        pc_tr(0, 0)
        pc_tr(0, 1)

        k = 0            # global attn tile index
        pend = None      # (g, j) whose PV is not yet emitted
        for g in range(NG):
            po_t[g] = ps_o.tile([HD + 1, 512], fp32, tag="po", name=f"po{g}")
            for j in range(g + 1):  # natural order, diagonal last
                attn_sx(g, j)
                if pend is not None:
                    attn_pv(*pend)
                pend = (g, j)
                if k == 0:
                    for half in range(2):
                        pc_q(1, half)
                    pc_kv(1)
                    pc_tr(1, 0)
                    pc_tr(1, 1)
                for piece in pieces.get(k, []):
                    piece()
                k += 1
        attn_pv(*pend)

    nc.compile()
    return nc


def _get_nc():
    if "nc" not in _cache:
        _cache["nc"] = _build_nc()
    return _cache["nc"]


def _perm(parity: int) -> np.ndarray:
    # chunk-local order: chunk c = [global block 2c+parity | block 2c+1-parity]
    blocks = np.arange(NKB).reshape(NG, 2)           # [[0,1],[2,3],...]
    if parity == 1:
        blocks = blocks[:, ::-1]
    return (blocks.reshape(-1)[:, None] * KB + np.arange(KB)[None, :]).ravel()


def _mask(parity: int) -> np.ndarray:
    r = np.arange(128)[:, None]
    j = np.arange(KB)[None, :]
    tri0 = (r <= j).astype(np.float32)            # key tile h=0 vs own block
    tri1 = (128 + r <= j).astype(np.float32)      # key tile h=1
    second = np.ones((128, KB), np.float32) if parity == 0 else np.zeros(
        (128, KB), np.float32)
    m = np.concatenate([tri0, second, tri1, second], axis=1)  # [128, 1024]
    return m.astype(BF16)


def _in_maps(x, Wq, Wk, Wv):
    wall = np.concatenate([Wk, Wv, Wq, Wq], axis=1).astype(BF16)
    masks = [_mask(0), _mask(1)]
    perm1 = _perm(1)
    in_maps = []
    for core in range(NCORES):
        b, par = core // 2, core % 2
        xTb = np.ascontiguousarray(x[b].T).astype(BF16)
        xT = xTb if par == 0 else np.ascontiguousarray(xTb[:, perm1])
        in_maps.append({"xT": xT, "wall": wall, "maskd": masks[par]})
    return in_maps


def _combine(outs):
    """outs: 8 arrays [65, T] fp32 -> full [B, T, HD] fp32."""
    full = np.empty((B, T, HD), np.float32)
    for b in range(B):
        oe = outs[2 * b]
        oo = outs[2 * b + 1].reshape(HD + 1, NG, 2, KB)[:, :, ::-1, :].reshape(
            HD + 1, T)
        num = oe[0:HD] + oo[0:HD]
        den = oe[HD] + oo[HD]
        full[b] = (num / den).T
    return full


def run(x, Wq, Wk, Wv, trace=False):
    from concourse.bass_utils import run_bass_kernel_spmd

    nc = _get_nc()
    in_maps = _in_maps(x, Wq, Wk, Wv)
    res = run_bass_kernel_spmd(
        nc, in_maps, core_ids=list(range(NCORES)), trace=trace,
    )
    outs = [r["out"] for r in res.results]
    return _combine(outs), res


def kernel(x, Wq, Wk, Wv, padding_mask=None, **_ignored):
    out, _ = run(np.asarray(x, np.float32), np.asarray(Wq, np.float32),
                 np.asarray(Wk, np.float32), np.asarray(Wv, np.float32))
    return out
